# revision 1
# baseline (speedup 1.0000x reference)
"""DTNN layer kernel for Trainium2 (8 NeuronCores).

Math: out[b,i,o] = sum_j sum_h Wfc[o,h] * hx[b,i,h] * hd[b,i,j,h]
with hx = x@Wcf.T + bcf, hd = dist@Wdf.T + bdf.
Since Wfc/Wdf are linear, the j-sum commutes:
    ds[b,i,d]  = sum_j dist[b,i,j,d]                  (memory-bound reduction)
    out[b,i,:] = ((x@Wcf.T + bcf) * (ds@Wdf.T + N*bdf)) @ Wfc.T
So the kernel streams `distance` once (134MB) and does a few 128x128 matmuls.

Sharding: flatten (B,N) -> 1024 i-rows, 128 rows per core; no cross-core comms.

Measured (NTFF profile, core 0): ~70us/core, vs ~47us pure HBM stream at the
358 GB/s per-core fair share plus ~13us fixed NEFF prologue/epilogue and a
~9us serial tail. Structure:
- dist is streamed as a few big HWDGE DMAs on one ring (in-order arrivals);
  DVE folds each tile to 128 columns in place right after it lands (halving
  unit-stride adds run at full DVE rate; strided reduces were 1.6x slower).
- biases are folded into PE matmuls as K=1 rank-1 updates, and the
  (hx * N*bdf) @ WfcT bias term is preloaded into the output PSUM during the
  stream so the post-stream tail is just transpose -> Wdf matmul -> mul ->
  accumulate-matmul -> store.
"""

import numpy as np

import concourse.bass as bass
import concourse.bacc as bacc
import concourse.mybir as mybir
from concourse.tile import TileContext
from concourse.bass_utils import run_bass_kernel_spmd

B, N, D, H = 4, 256, 128, 128
NCORES = 8
ROWS = B * N // NCORES  # 128 i-rows per core
FP = mybir.dt.float32

# packed constant columns: [xT | wcfT | wdfT | wfcT | eye | rows...]
C_XT = 0
C_WCF = 128
C_WDF = 256
C_WFC = 384
C_EYE = 512
C_BCFR = 640   # partition 0: bcf row (1, H)
C_BDFR = 768   # partition 0: bdf row (1, H)
C_ONES = 896   # partition 0: ones row (1, ROWS)
C_BDFC = 1024  # bdf as a per-partition column (H, 1)
C_TOT = 1025


def build_nc():
    nc = bacc.Bacc("TRN2", target_bir_lowering=False)
    dist = nc.declare_dram_parameter("dist", [ROWS, N * D], FP, isOutput=False)
    cst = nc.declare_dram_parameter("cst", [128, C_TOT], FP, isOutput=False)
    out = nc.declare_dram_parameter("out", [ROWS, D], FP, isOutput=True)

    with TileContext(nc) as tc:
        with (
            tc.tile_pool(name="const", bufs=1) as cpool,
            tc.tile_pool(name="dist", bufs=1) as dpool,
            tc.tile_pool(name="work", bufs=1) as wpool,
            tc.tile_pool(name="psum", bufs=1, space="PSUM") as ppool,
        ):
            # Issue the dist stream first so the big DMAs start ASAP; the
            # constants ride behind them on the same queue.
            SIZES = [64, 64, 64, 32, 16, 8, 4, 4]  # j-counts per DMA tile
            dtiles = []
            off = 0
            for k, jn in enumerate(SIZES):
                t = dpool.tile([ROWS, jn * D], FP, tag=f"dist{k}")
                # Single HWDGE ring (SP): in-order arrivals matching the DVE
                # fold order; the stream is HBM-fair-share-bound (~358GB/s)
                # so a second ring adds no bandwidth, only ordering jitter.
                nc.sync.dma_start(out=t[:], in_=dist[:, off * D:(off + jn) * D])
                dtiles.append(t)
                off += jn

            cst_t = cpool.tile([128, C_TOT], FP)
            nc.scalar.dma_start(out=cst_t[:], in_=cst[:])
            xT_t = cst_t[:, C_XT:C_XT + ROWS]
            wcf_t = cst_t[:, C_WCF:C_WCF + H]
            wdf_t = cst_t[:, C_WDF:C_WDF + H]
            wfc_t = cst_t[:, C_WFC:C_WFC + D]
            ident = cst_t[:, C_EYE:C_EYE + ROWS]
            bcf_row = cst_t[0:1, C_BCFR:C_BCFR + H]
            ones_row = cst_t[0:1, C_ONES:C_ONES + ROWS]

            # hx^T = (Wcf^T)^T @ x^T + bcf x ones -> (H, ROWS) in PSUM
            hx_ps = ppool.tile([H, ROWS], FP)
            nc.tensor.matmul(hx_ps[:], wcf_t, xT_t, start=True, stop=False)
            nc.tensor.matmul(hx_ps[:], bcf_row, ones_row, start=False, stop=True)
            hxT = wpool.tile([H, ROWS], FP)
            nc.vector.tensor_copy(hxT[:], hx_ps[:])

            # Preload the bias term (hx * N*bdf) @ Wfc^T into the output
            # PSUM during the stream; the tail's out-matmul accumulates
            # onto it, removing the bias matmul from the critical tail.
            bdfN = wpool.tile([H, 1], FP)
            nc.vector.tensor_scalar_mul(bdfN[:], cst_t[:, C_BDFC:C_BDFC + 1],
                                        float(N))
            s0T = wpool.tile([H, ROWS], FP)
            nc.vector.tensor_scalar_mul(s0T[:], hxT[:], bdfN[:])
            out_ps = ppool.tile([ROWS, D], FP)
            nc.tensor.matmul(out_ps[:], s0T[:], wfc_t, start=True, stop=False)

            # Streaming j-reduction: ds[i,d] = sum_j dist[i,j,d].
            # Each tile is folded to 128 columns in place immediately after
            # its DMA lands (halving adds, all unit-stride = full DVE rate),
            # then added into the running accumulator (tile 0). Per-tile DVE
            # work (~4.9us) keeps pace with per-tile DMA arrival (~5.1us),
            # so only ~2us of DVE work remains after the last (half-size)
            # tile arrives.
            acc = dtiles[0]
            for k, jn in enumerate(SIZES):
                t = dtiles[k]
                half = jn * D // 2
                while half >= D:
                    nc.vector.tensor_add(
                        t[:, 0:half], t[:, 0:half], t[:, half:2 * half]
                    )
                    half //= 2
                if k > 0:
                    nc.vector.tensor_add(acc[:, 0:D], acc[:, 0:D], t[:, 0:D])
            ds = acc[:, 0:D]

            # ds (i,d) -> dsT (d,i) via PE transpose
            dsT_ps = ppool.tile([D, ROWS], FP)
            nc.tensor.transpose(dsT_ps[:], ds, ident)
            dsT = wpool.tile([D, ROWS], FP)
            nc.vector.tensor_copy(dsT[:], dsT_ps[:])

            # hd^T (bias-free) = (Wdf^T)^T @ ds^T -> (H, ROWS)
            hd_ps = ppool.tile([H, ROWS], FP)
            nc.tensor.matmul(hd_ps[:], wdf_t, dsT[:], start=True, stop=True)

            # s^T = hx^T * hd^T (one PSUM operand max per DVE op)
            sT = wpool.tile([H, ROWS], FP)
            nc.vector.tensor_mul(sT[:], hd_ps[:], hxT[:])

            # out += sT^T @ Wfc^T, accumulating onto the preloaded bias term
            nc.tensor.matmul(out_ps[:], sT[:], wfc_t, start=False, stop=True,
                             skip_group_check=True)
            out_sb = wpool.tile([ROWS, D], FP)
            nc.vector.tensor_copy(out_sb[:], out_ps[:])
            nc.sync.dma_start(out=out[:], in_=out_sb[:])
    nc.compile()
    return nc


_NC_CACHE = None


def _get_nc():
    global _NC_CACHE
    if _NC_CACHE is None:
        _NC_CACHE = build_nc()
    return _NC_CACHE


def _make_in_maps(x, distance, Wcf_w, Wcf_b, Wdf_w, Wdf_b, Wfc_w):
    x = np.ascontiguousarray(np.asarray(x, np.float32))
    distance = np.ascontiguousarray(np.asarray(distance, np.float32))
    x_flat = x.reshape(B * N, D)
    dist_flat = distance.reshape(B * N, N * D)
    wcfT = np.asarray(Wcf_w, np.float32).T
    wdfT = np.asarray(Wdf_w, np.float32).T
    wfcT = np.asarray(Wfc_w, np.float32).T
    bcf = np.asarray(Wcf_b, np.float32)
    bdf = np.asarray(Wdf_b, np.float32)
    in_maps = []
    for c in range(NCORES):
        sl = slice(c * ROWS, (c + 1) * ROWS)
        cstblk = np.zeros((128, C_TOT), np.float32)
        cstblk[:, C_XT:C_XT + ROWS] = x_flat[sl].T
        cstblk[:, C_WCF:C_WCF + H] = wcfT
        cstblk[:, C_WDF:C_WDF + H] = wdfT
        cstblk[:, C_WFC:C_WFC + D] = wfcT
        cstblk[:, C_EYE:C_EYE + ROWS] = np.eye(ROWS, dtype=np.float32)
        cstblk[0, C_BCFR:C_BCFR + H] = bcf
        cstblk[0, C_BDFR:C_BDFR + H] = bdf
        cstblk[0, C_ONES:C_ONES + ROWS] = 1.0
        cstblk[:, C_BDFC] = bdf
        in_maps.append({
            "dist": np.ascontiguousarray(dist_flat[sl]),
            "cst": cstblk,
        })
    return in_maps


def kernel(x, distance, Wcf_w, Wcf_b, Wdf_w, Wdf_b, Wfc_w):
    in_maps = _make_in_maps(x, distance, Wcf_w, Wcf_b, Wdf_w, Wdf_b, Wfc_w)
    nc = _get_nc()
    res = run_bass_kernel_spmd(nc, in_maps, list(range(NCORES))).results
    out = np.concatenate([res[c]["out"] for c in range(NCORES)], axis=0)
    return out.reshape(B, N, D)



# revision 3
# speedup vs baseline: 1.4033x; 1.4033x over previous
"""DTNN layer kernel for Trainium2 (8 NeuronCores).

Math: out[b,i,o] = sum_j sum_h Wfc[o,h] * hx[b,i,h] * hd[b,i,j,h]
with hx = x@Wcf.T + bcf, hd = dist@Wdf.T + bdf.
Since Wfc/Wdf are linear, the j-sum commutes:
    ds[b,i,d]  = sum_j dist[b,i,j,d]                  (memory-bound reduction)
    out[b,i,:] = ((x@Wcf.T + bcf) * (ds@Wdf.T + N*bdf)) @ Wfc.T
So the kernel streams `distance` once and does a few 128x128 matmuls.

Sharding: flatten (B,N) -> 1024 i-rows, 128 rows per core; no cross-core comms.

v2 design (from NTFF trace analysis of the fp32 baseline):
- distance is cast to fp16 on the host (tolerance is 2e-2; fp16 keeps the
  result at ~1e-3): halves the HBM stream and doubles DVE fold throughput.
- Host lays the per-core shard out as [d, j, i] so the in-SBUF halving
  fold over j directly yields ds^T = [d, i]; the tail is then just
  hd^T = (Wdf^T)^T @ ds^T -> mul by hx^T -> accumulate-matmul -> store,
  with no PE transpose and fewer PSUM round-trips.
- Constants go FIRST on the same HWDGE queue as the stream: in the
  baseline they sat on a second queue that packet-round-robins against
  32KB stream packets (44 GB/s), stalling the whole DVE program (which
  starts with the hx chain) until t=24us.
- Tile sizes ascend [4,8,...,64] so fold work starts as soon as ~10us,
  and a tiny 4-j tile lands last so the post-stream serial tail is short.
- The (hx * N*bdf) @ WfcT bias term is preloaded into the output PSUM
  during the stream; N*bdf is prescaled on the host.
"""

import numpy as np

import concourse.bass as bass
import concourse.bacc as bacc
import concourse.mybir as mybir
from concourse.tile import TileContext
from concourse.bass_utils import run_bass_kernel_spmd

B, N, D, H = 4, 256, 128, 128
NCORES = 8
ROWS = B * N // NCORES  # 128 i-rows per core
FP = mybir.dt.float32
F16 = mybir.dt.float16

# dist DRAM layout per core: [128 d-partitions, N*ROWS cols], col = j*ROWS + i
SIZES = [4, 8, 16, 32, 64, 64, 64, 4]  # j-counts per DMA tile (all pow2)

# cst16 columns: [xT | wcfT | wdfT | rows(bcf/ones)]
C16_XT = 0
C16_WCF = 128
C16_WDF = 256
C16_BCF = 384   # partition 0: bcf row
C16_ONES = 512  # partition 0: ones row
C16_TOT = 640
# cst32 columns: [wfcT | N*bdf col]
C32_WFC = 0
C32_BDFN = 128
C32_TOT = 129


def build_nc():
    nc = bacc.Bacc("TRN2", target_bir_lowering=False)
    dist = nc.declare_dram_parameter("dist", [128, N * ROWS], F16, isOutput=False)
    cst16 = nc.declare_dram_parameter("cst16", [128, C16_TOT], F16, isOutput=False)
    cst32 = nc.declare_dram_parameter("cst32", [128, C32_TOT], FP, isOutput=False)
    out = nc.declare_dram_parameter("out", [ROWS, D], FP, isOutput=True)

    with TileContext(nc) as tc:
        with (
            tc.tile_pool(name="const", bufs=1) as cpool,
            tc.tile_pool(name="dist", bufs=1) as dpool,
            tc.tile_pool(name="work", bufs=1) as wpool,
            tc.tile_pool(name="psum", bufs=1, space="PSUM") as ppool,
        ):
            # Constants first on the stream queue: ~0.5us, then the big
            # dist tiles follow on the same HWDGE ring (in-order arrivals).
            c16 = cpool.tile([128, C16_TOT], F16)
            nc.sync.dma_start(out=c16[:], in_=cst16[:])
            c32 = cpool.tile([128, C32_TOT], FP)
            nc.sync.dma_start(out=c32[:], in_=cst32[:])

            dtiles = []
            off = 0
            for k, jn in enumerate(SIZES):
                t = dpool.tile([128, jn * ROWS], F16, tag=f"dist{k}")
                nc.sync.dma_start(out=t[:], in_=dist[:, off * ROWS:(off + jn) * ROWS])
                dtiles.append(t)
                off += jn

            xT = c16[:, C16_XT:C16_XT + ROWS]
            wcf = c16[:, C16_WCF:C16_WCF + H]
            wdf = c16[:, C16_WDF:C16_WDF + H]
            bcf_row = c16[0:1, C16_BCF:C16_BCF + H]
            ones_row = c16[0:1, C16_ONES:C16_ONES + ROWS]
            wfc = c32[:, C32_WFC:C32_WFC + D]
            bdfN = c32[:, C32_BDFN:C32_BDFN + 1]

            # hx^T = (Wcf^T)^T @ x^T + bcf x ones -> (H, ROWS) in PSUM
            hx_ps = ppool.tile([H, ROWS], FP)
            nc.tensor.matmul(hx_ps[:], wcf, xT, start=True, stop=False)
            nc.tensor.matmul(hx_ps[:], bcf_row, ones_row, start=False, stop=True)
            hxT = wpool.tile([H, ROWS], FP)
            nc.vector.tensor_copy(hxT[:], hx_ps[:])

            # Preload the bias term (hx * N*bdf) @ Wfc^T into the output
            # PSUM during the stream; the tail's out-matmul accumulates on it.
            s0T = wpool.tile([H, ROWS], FP)
            nc.vector.tensor_scalar_mul(s0T[:], hxT[:], bdfN)
            out_ps = ppool.tile([ROWS, D], FP)
            nc.tensor.matmul(out_ps[:], s0T[:], wfc, start=True, stop=False)

            # Streaming j-reduction in fp16: each tile [128 d, jn*ROWS] is
            # viewed as jn blocks of ROWS columns; halving adds (contiguous,
            # 2x DVE mode) fold it to one [128, ROWS] block = partial ds^T.
            acc = dtiles[0]
            for k, jn in enumerate(SIZES):
                t = dtiles[k]
                half = jn // 2
                while half >= 1:
                    nc.vector.tensor_add(
                        t[:, 0:half * ROWS],
                        t[:, 0:half * ROWS],
                        t[:, half * ROWS:2 * half * ROWS],
                    )
                    half //= 2
                if k > 0:
                    nc.vector.tensor_add(
                        acc[:, 0:ROWS], acc[:, 0:ROWS], t[:, 0:ROWS]
                    )
            dsT = acc[:, 0:ROWS]  # (128 d, ROWS i) fp16

            # hd^T (bias-free) = (Wdf^T)^T @ ds^T -> (H, ROWS), fp16 matmul
            hd_ps = ppool.tile([H, ROWS], FP)
            nc.tensor.matmul(hd_ps[:], wdf, dsT, start=True, stop=True)

            # s^T = hd^T * hx^T (one PSUM operand max per DVE op)
            sT = wpool.tile([H, ROWS], FP)
            nc.vector.tensor_mul(sT[:], hd_ps[:], hxT[:])

            # out += s^T^T @ Wfc^T, accumulating onto the preloaded bias term
            nc.tensor.matmul(out_ps[:], sT[:], wfc, start=False, stop=True,
                             skip_group_check=True)
            out_sb = wpool.tile([ROWS, D], FP)
            nc.vector.tensor_copy(out_sb[:], out_ps[:])
            nc.sync.dma_start(out=out[:], in_=out_sb[:])
    nc.compile()
    return nc


_NC_CACHE = None


def _get_nc():
    global _NC_CACHE
    if _NC_CACHE is None:
        _NC_CACHE = build_nc()
    return _NC_CACHE


def _make_in_maps(x, distance, Wcf_w, Wcf_b, Wdf_w, Wdf_b, Wfc_w):
    x = np.asarray(x, np.float32)
    x_flat = x.reshape(B * N, D)
    # [B*N, N, D] -> fp16 -> [d, j, i_full] once, then slice per core
    d16 = np.asarray(distance, np.float32).astype(np.float16)
    dT = np.ascontiguousarray(d16.reshape(B * N, N, D).transpose(2, 1, 0))
    wcfT = np.asarray(Wcf_w, np.float32).T
    wdfT = np.asarray(Wdf_w, np.float32).T
    wfcT = np.asarray(Wfc_w, np.float32).T
    bcf = np.asarray(Wcf_b, np.float32)
    bdf = np.asarray(Wdf_b, np.float32)
    in_maps = []
    for c in range(NCORES):
        sl = slice(c * ROWS, (c + 1) * ROWS)
        c16blk = np.zeros((128, C16_TOT), np.float16)
        c16blk[:, C16_XT:C16_XT + ROWS] = x_flat[sl].T
        c16blk[:, C16_WCF:C16_WCF + H] = wcfT
        c16blk[:, C16_WDF:C16_WDF + H] = wdfT
        c16blk[0, C16_BCF:C16_BCF + H] = bcf
        c16blk[0, C16_ONES:C16_ONES + ROWS] = 1.0
        c32blk = np.zeros((128, C32_TOT), np.float32)
        c32blk[:, C32_WFC:C32_WFC + D] = wfcT
        c32blk[:, C32_BDFN] = float(N) * bdf
        in_maps.append({
            "dist": np.ascontiguousarray(dT[:, :, sl]).reshape(128, N * ROWS),
            "cst16": c16blk,
            "cst32": c32blk,
        })
    return in_maps


def kernel(x, distance, Wcf_w, Wcf_b, Wdf_w, Wdf_b, Wfc_w):
    in_maps = _make_in_maps(x, distance, Wcf_w, Wcf_b, Wdf_w, Wdf_b, Wfc_w)
    nc = _get_nc()
    res = run_bass_kernel_spmd(nc, in_maps, list(range(NCORES))).results
    out = np.concatenate([res[c]["out"] for c in range(NCORES)], axis=0)
    return out.reshape(B, N, D)


# revision 4
# speedup vs baseline: 1.8972x; 1.3520x over previous
"""DTNN layer kernel for Trainium2 (8 NeuronCores).

Math: out[b,i,o] = sum_j sum_h Wfc[o,h] * hx[b,i,h] * hd[b,i,j,h]
with hx = x@Wcf.T + bcf, hd = dist@Wdf.T + bdf.
Since Wfc/Wdf are linear, the j-sum commutes:
    ds[b,i,d]  = sum_j dist[b,i,j,d]                  (memory-bound reduction)
    out[b,i,:] = ((x@Wcf.T + bcf) * (ds@Wdf.T + N*bdf)) @ Wfc.T
So the kernel streams `distance` once and does a few 128x128 matmuls.

Sharding: flatten (B,N) -> 1024 i-rows, 128 rows per core; no cross-core comms.

v3 design (from NTFF trace analysis of v1/v2):
- distance is cast to fp16 on the host (tolerance is 2e-2; result stays at
  ~1e-3): halves the HBM stream and doubles DVE throughput.
- Host lays the per-core shard out as [d, j, i]; a partial in-SBUF halving
  fold over j yields 8 blocks of partial ds^T per tile.
- The j-sum commutes through Wdf, so the remaining reduction rides the
  (otherwise idle) PE: each 128-col block is one fp16 accumulating matmul
  into hd_ps.  This caps DVE busy (~21us) below the stream span (~24us);
  in v2 a full DVE fold (28.5us busy at the measured 0.625ns/elem TT rate)
  lagged the stream by ~13us.
- Constants ride FIRST on the stream HWDGE queue (a second queue gets
  starved by packet round-robin against big stream packets).
- Big 64-j tiles stream early so fold+matmul work overlaps arrival; a tiny
  4-j tile lands last so the post-stream serial tail is short.
- All-fp16 tail: hx^T is kept in fp16, the bias term uses a host-folded
  N*bdf[h]*Wfc[h,o] matrix, and the final out-matmul is fp16 single-pass
  (the v2 fp32 LOW_HIGH out-matmul cost two PE passes).
"""

import numpy as np

import concourse.bass as bass
import concourse.bacc as bacc
import concourse.mybir as mybir
from concourse.tile import TileContext
from concourse.bass_utils import run_bass_kernel_spmd

B, N, D, H = 4, 256, 128, 128
NCORES = 8
ROWS = B * N // NCORES  # 128 i-rows per core
FP = mybir.dt.float32
F16 = mybir.dt.float16

# dist DRAM layout per core: [128 d-partitions, N*ROWS cols], col = j*ROWS + i
SIZES = [4, 64, 64, 64, 32, 16, 8, 4]  # j-counts per DMA tile (all pow2)

# cst16 columns (all fp16)
C16_XT = 0      # x^T            (128 d, ROWS i)
C16_WCF = 128   # Wcf^T          (128 d, H)
C16_WDF = 256   # Wdf^T          (128 d, H)
C16_BCF = 384   # partition 0: bcf row (1, H)
C16_ONES = 512  # partition 0: ones row (1, ROWS)
C16_WFC = 640   # Wfc^T          (128 h, D)
C16_WFCB = 768  # N*bdf[h] * Wfc^T[h,o]  (128 h, D)
C16_TOT = 896


def build_nc():
    nc = bacc.Bacc("TRN2", target_bir_lowering=False)
    dist = nc.declare_dram_parameter("dist", [128, N * ROWS], F16, isOutput=False)
    cst16 = nc.declare_dram_parameter("cst16", [128, C16_TOT], F16, isOutput=False)
    out = nc.declare_dram_parameter("out", [ROWS, D], FP, isOutput=True)

    with TileContext(nc) as tc:
        with (
            tc.tile_pool(name="const", bufs=1) as cpool,
            tc.tile_pool(name="dist", bufs=1) as dpool,
            tc.tile_pool(name="work", bufs=1) as wpool,
            tc.tile_pool(name="psum", bufs=1, space="PSUM") as ppool,
        ):
            # Constants first on the stream queue (~0.6us), then the big
            # dist tiles follow on the same HWDGE ring (in-order arrivals).
            c16 = cpool.tile([128, C16_TOT], F16)
            nc.sync.dma_start(out=c16[:], in_=cst16[:])

            dtiles = []
            off = 0
            for k, jn in enumerate(SIZES):
                t = dpool.tile([128, jn * ROWS], F16, tag=f"dist{k}")
                nc.sync.dma_start(out=t[:], in_=dist[:, off * ROWS:(off + jn) * ROWS])
                dtiles.append(t)
                off += jn

            xT = c16[:, C16_XT:C16_XT + ROWS]
            wcf = c16[:, C16_WCF:C16_WCF + H]
            wdf = c16[:, C16_WDF:C16_WDF + H]
            bcf_row = c16[0:1, C16_BCF:C16_BCF + H]
            ones_row = c16[0:1, C16_ONES:C16_ONES + ROWS]
            wfc16 = c16[:, C16_WFC:C16_WFC + D]
            wfcb16 = c16[:, C16_WFCB:C16_WFCB + D]

            # hx^T = (Wcf^T)^T @ x^T + bcf x ones -> (H, ROWS) in PSUM,
            # kept in fp16 for the all-fp16 tail matmuls.
            hx_ps = ppool.tile([H, ROWS], FP)
            nc.tensor.matmul(hx_ps[:], wcf, xT, start=True, stop=False)
            nc.tensor.matmul(hx_ps[:], bcf_row, ones_row, start=False, stop=True)
            hxT = wpool.tile([H, ROWS], F16)
            nc.vector.tensor_copy(hxT[:], hx_ps[:])

            # Preload the bias term hx^T @ (N*bdf*Wfc^T) into the output
            # PSUM during the stream; the tail's out-matmul accumulates on it.
            out_ps = ppool.tile([ROWS, D], FP)
            nc.tensor.matmul(out_ps[:], hxT[:], wfcb16, start=True, stop=False)

            # Streaming j-reduction: each tile [128 d, jn*ROWS] is jn blocks
            # of ROWS columns.  DVE halving adds (2x fp16 mode) fold big
            # tiles down to 8 blocks; each remaining block is one fp16
            # accumulating matmul into hd_ps on the otherwise-idle PE
            # (sum_j commutes through Wdf).  Small tiles fold to 1 block.
            hd_ps = ppool.tile([H, ROWS], FP)
            n_mms = sum(8 if jn >= 16 else 1 for jn in SIZES)
            mi = 0
            for k, jn in enumerate(SIZES):
                t = dtiles[k]
                nblk = 8 if jn >= 16 else 1
                half = jn // 2
                while half >= nblk:
                    nc.vector.tensor_add(
                        t[:, 0:half * ROWS],
                        t[:, 0:half * ROWS],
                        t[:, half * ROWS:2 * half * ROWS],
                    )
                    half //= 2
                for b in range(nblk):
                    nc.tensor.matmul(
                        hd_ps[:], wdf, t[:, b * ROWS:(b + 1) * ROWS],
                        start=(mi == 0), stop=(mi == n_mms - 1),
                    )
                    mi += 1

            # s^T = hd^T * hx^T (one PSUM operand max per DVE op), fp16
            sT = wpool.tile([H, ROWS], F16)
            nc.vector.tensor_mul(sT[:], hd_ps[:], hxT[:])

            # out += s^T^T @ Wfc^T (fp16 single pass), onto the bias term
            nc.tensor.matmul(out_ps[:], sT[:], wfc16, start=False, stop=True,
                             skip_group_check=True)
            out_sb = wpool.tile([ROWS, D], FP)
            nc.vector.tensor_copy(out_sb[:], out_ps[:])
            nc.sync.dma_start(out=out[:], in_=out_sb[:])
    nc.compile()
    return nc


_NC_CACHE = None


def _get_nc():
    global _NC_CACHE
    if _NC_CACHE is None:
        _NC_CACHE = build_nc()
    return _NC_CACHE


def _make_in_maps(x, distance, Wcf_w, Wcf_b, Wdf_w, Wdf_b, Wfc_w):
    x = np.asarray(x, np.float32)
    x_flat = x.reshape(B * N, D)
    # [B*N, N, D] -> fp16 -> [d, j, i_full] once, then slice per core
    d16 = np.asarray(distance, np.float32).astype(np.float16)
    dT = np.ascontiguousarray(d16.reshape(B * N, N, D).transpose(2, 1, 0))
    wcfT = np.asarray(Wcf_w, np.float32).T
    wdfT = np.asarray(Wdf_w, np.float32).T
    wfcT = np.asarray(Wfc_w, np.float32).T
    bcf = np.asarray(Wcf_b, np.float32)
    bdf = np.asarray(Wdf_b, np.float32)
    wfcb = (float(N) * bdf)[:, None] * wfcT  # (h, o)
    in_maps = []
    for c in range(NCORES):
        sl = slice(c * ROWS, (c + 1) * ROWS)
        c16blk = np.zeros((128, C16_TOT), np.float16)
        c16blk[:, C16_XT:C16_XT + ROWS] = x_flat[sl].T
        c16blk[:, C16_WCF:C16_WCF + H] = wcfT
        c16blk[:, C16_WDF:C16_WDF + H] = wdfT
        c16blk[0, C16_BCF:C16_BCF + H] = bcf
        c16blk[0, C16_ONES:C16_ONES + ROWS] = 1.0
        c16blk[:, C16_WFC:C16_WFC + D] = wfcT
        c16blk[:, C16_WFCB:C16_WFCB + D] = wfcb
        in_maps.append({
            "dist": np.ascontiguousarray(dT[:, :, sl]).reshape(128, N * ROWS),
            "cst16": c16blk,
        })
    return in_maps


def kernel(x, distance, Wcf_w, Wcf_b, Wdf_w, Wdf_b, Wfc_w):
    in_maps = _make_in_maps(x, distance, Wcf_w, Wcf_b, Wdf_w, Wdf_b, Wfc_w)
    nc = _get_nc()
    res = run_bass_kernel_spmd(nc, in_maps, list(range(NCORES))).results
    out = np.concatenate([res[c]["out"] for c in range(NCORES)], axis=0)
    return out.reshape(B, N, D)
